# revision 29
# baseline (speedup 1.0000x reference)
"""NPS (non-printability score) kernel for Trainium2, 8-core data-parallel.

Math: for each pixel x (3 channels), distance to each of 30 printability
colors p_k is  d2_k = sum_c (x_c - p_c + 1e-6)^2 + 1e-6.  The score is
sum over pixels of sqrt(min_k d2_k), divided by adv_patch.size.

With q = p - 1e-6:  d2_k = S + (-2 x.q_k) + (T_k + 1e-6) where S = sum
x_c^2, T_k = |q_k|^2.  fp16 matmuls (1 PE cycle/column vs fp32's 4)
compute d2 for 8 colors x 16 pixel groups per 512-column pass; 4 passes
cover the 32 (padded) colors.  T rides in as two "ones"-row weights
(split hi/lo over two fp16 rows to kill weight-rounding error); the
ones rows and x rows arrive in a single HBM load per slab.

Post-matmul min funnel (z is fp32 in PSUM - TRN2 matmuls can't narrow -
and the funnel is boxed in by four hardware rules: tensor_tensor may
read at most one PSUM operand, GPSIMD has no PSUM access and no min op,
DMA cannot touch PSUM, and engine operands must start on 32-aligned
partitions.  Every z value therefore crosses PSUM->SBUF exactly once,
split across the two engines that can do it):
  - ScalarE: passes 0,1 (per-slot ops on a 3-bank rotation so PSUM
    frees early) and pass 2 (pair-merged) -> fp16 SBUF via Relu, which
    also clamps fp16-rounding negatives.
  - DVE: two packed-fp16 mins (2x rate) fold the converted passes, one
    mixed min chains pass 3 straight from PSUM.
  - PE transposes the survivor (fp16; colors k land packed innermost in
    each transposed block since lhsT columns are g*8+k); DVE folds the
    8 colors with a copy + packed-fp16 min tree (one PSUM half is
    copied out first so every TT sees at most one PSUM operand).
  - Per-pixel minima collect into a [128, 8*128] tile per 8 pairs; a
    tensor_scalar max-0 clamp and one ScalarE sqrt+accumulate per
    collector finish the job.  GPSIMD computes the x^2 rhs rows
    (Multiply is in its op set); a dozen warm-up matmuls ahead of the
    first real one hold the PE p-state at full clock.

PSUM (8 banks): z01 rotation 3 + z2 [128,2,512] 2 + z3 [128,2,512] 2
+ pt transpose buffer 1.

Sharding: batch dim (8 images) -> 8 NeuronCores, printability replicated.
"""

import numpy as np

import concourse.bass as bass
import concourse.bacc as bacc
import concourse.tile as tile
import concourse.mybir as mybir
from concourse.bass_utils import run_bass_kernel_spmd

F32 = mybir.dt.float32
F16 = mybir.dt.float16
I32 = mybir.dt.int32
ALU = mybir.AluOpType
ACTF = mybir.ActivationFunctionType

B, C, H, W = 8, 3, 512, 512
NCOLORS = 30
NPAD = 32            # colors padded to 32
NPASS = 4            # color passes, 8 colors each
CPP = 8              # colors per pass
G = 16               # pixel groups per matmul column block
MMN = 512            # matmul moving free dim (one fp32 PSUM bank)
NFREE = 4096         # per-partition free size of one slab
NSLAB = 4            # 4 slabs x 16 groups x 4096 = 262144 pixels/core
STS = NFREE // MMN   # supertiles per slab = 8
NPAIR = NSLAB * STS // 2   # 16 parity pairs
X0 = 64              # x rows base in rhs (rows 50..63 are zero pad)
ONES0 = 48           # two ones rows (T_hi / T_lo weights)
ROWS = 112
EPS = 1e-6
TBIG = 60000.0       # padded-color T: huge but finite in fp16


def _build_program(probe=None):
    nc = bacc.Bacc(
        "TRN2",
        target_bir_lowering=False,
        debug=False,
        enable_asserts=False,
        num_devices=B,
    )
    # x slab rows: 0..1 ones, 2..15 zero pad, 16..63 x (c*16+g)
    x_d = nc.dram_tensor("x", [NSLAB, 64, NFREE], F16, kind="ExternalInput")
    p_d = nc.dram_tensor("p", [NCOLORS, C], F32, kind="ExternalInput")
    out_d = nc.dram_tensor("out", [128, 2], F32, kind="ExternalOutput")

    with tile.TileContext(nc) as tc:
        _body(tc, nc, x_d, p_d, out_d, probe)
    nc.compile()
    return nc


def _body(tc, nc, x_d, p_d, out_d, probe=None):
    import contextlib

    ctx = contextlib.ExitStack()
    const = ctx.enter_context(tc.tile_pool(name="const", bufs=1))
    spool = ctx.enter_context(tc.tile_pool(name="spool", bufs=3))
    mpool = ctx.enter_context(tc.tile_pool(name="mpool", bufs=3))
    cpool = ctx.enter_context(tc.tile_pool(name="cpool", bufs=3))
    s2pool = ctx.enter_context(tc.tile_pool(name="s2pool", bufs=3))
    stpool = ctx.enter_context(tc.tile_pool(name="stpool", bufs=4))
    collp = ctx.enter_context(tc.tile_pool(name="collp", bufs=2))
    t1pool = ctx.enter_context(tc.tile_pool(name="t1pool", bufs=2))
    sqp = ctx.enter_context(tc.tile_pool(name="sqp", bufs=2))
    zpool = ctx.enter_context(tc.tile_pool(name="zpool", bufs=3, space="PSUM"))
    z2pool = ctx.enter_context(tc.tile_pool(name="z2pool", bufs=1, space="PSUM"))
    z3pool = ctx.enter_context(tc.tile_pool(name="z3pool", bufs=1, space="PSUM"))
    ptpool = ctx.enter_context(tc.tile_pool(name="ptpool", bufs=1, space="PSUM"))

    # ---------------- preamble: constants -------------------------------
    czero = const.tile([128, 1], F32)
    nc.vector.memset(czero, 0.0)
    nc.const_aps.aps[(F32, 0.0)] = czero[:]


    psbt = const.tile([1, C, NCOLORS], F32)
    hp = tc.high_priority()
    hp.__enter__()
    nc.sync.dma_start(out=psbt, in_=p_d.ap().transpose([1, 0]).unsqueeze(0))
    # rhs buffers: rows 0..47 squares, 48..49 ones + 50..63 pad (row 52
    # carries the fp16 printability table in slab 0) + 64..111 x
    rhs_bufs = []
    for i in range(3):
        rhs = const.tile([ROWS, NFREE], F16, tag=f"rhs{i}")
        rhs_bufs.append(rhs)
    for s in range(NSLAB):
        eng = nc.sync if s % 2 == 0 else nc.scalar
        eng.dma_start(out=rhs_bufs[s % 3][ONES0:ROWS, :], in_=x_d.ap()[s])

    # squares all on GPSIMD (Multiply is in its supported op set),
    # in quarters so nothing blocks the Pool stream for long
    def emit_square(s, quarter=None):
        # slab 0 runs on DVE: the Pool list-scheduler orders its stream by
        # its own heuristic and reliably starves the first slab otherwise
        eng = nc.vector if s == 0 else nc.gpsimd
        rhs = rhs_bufs[s % 3]
        qs = [quarter] if quarter is not None else range(4)
        for q in qs:
            q0 = q * (NFREE // 4)
            sl = slice(q0, q0 + NFREE // 4)
            eng.tensor_tensor(
                out=rhs[0:48, sl], in0=rhs[X0:ROWS, sl], in1=rhs[X0:ROWS, sl],
                op=ALU.mult,
            )


    # identity 128x128 fp16 for PE transpose
    iop128 = const.tile([128, 1], I32)
    nc.gpsimd.iota(iop128, pattern=[[0, 1]], base=0, channel_multiplier=1)
    iof128 = const.tile([128, 128], I32)
    nc.gpsimd.iota(iof128, pattern=[[1, 128]], base=0, channel_multiplier=0)
    id128 = const.tile([128, 128], F16)
    nc.vector.tensor_tensor(
        out=id128, in0=iof128, in1=iop128.to_broadcast([128, 128]), op=ALU.is_equal
    )

    # sten[p, g] = ((p & 15) == g); ones rows 48..49 forced to 1 for all g
    iop112 = const.tile([ROWS, 1], I32)
    nc.gpsimd.iota(iop112, pattern=[[0, 1]], base=0, channel_multiplier=1)
    pm112 = const.tile([ROWS, 1], I32)
    nc.vector.tensor_scalar(
        out=pm112, in0=iop112, scalar1=15, scalar2=None, op0=ALU.bitwise_and
    )
    iof16 = const.tile([ROWS, G], I32)
    nc.gpsimd.iota(iof16, pattern=[[1, G]], base=0, channel_multiplier=0)
    sten = const.tile([ROWS, G], F32)
    nc.vector.tensor_tensor(
        out=sten, in0=iof16, in1=pm112.to_broadcast([ROWS, G]), op=ALU.is_equal
    )
    mo_ge = const.tile([ROWS, 1], I32)
    nc.vector.tensor_scalar(out=mo_ge, in0=iop112, scalar1=ONES0 - 1,
                            scalar2=None, op0=ALU.is_gt)
    mo_lt = const.tile([ROWS, 1], I32)
    nc.vector.tensor_scalar(out=mo_lt, in0=iop112, scalar1=ONES0 + 2,
                            scalar2=None, op0=ALU.is_lt)
    mo = const.tile([ROWS, 1], F32)
    nc.vector.tensor_tensor(out=mo, in0=mo_ge, in1=mo_lt, op=ALU.mult)
    nc.vector.tensor_tensor(out=sten, in0=sten,
                            in1=mo.to_broadcast([ROWS, G]), op=ALU.max)

    # ---------------- preamble: weight table ----------------------------
    # q = p - eps;  T_k = |q_k|^2 + eps  (split hi/lo over two fp16 rows)
    qt = const.tile([1, C, NCOLORS], F32)
    nc.vector.tensor_scalar(out=qt, in0=psbt, scalar1=-EPS, scalar2=None,
                            op0=ALU.add)
    q2 = const.tile([1, C, NCOLORS], F32)
    nc.vector.tensor_tensor(out=q2, in0=qt, in1=qt, op=ALU.mult)
    tsum = const.tile([1, NCOLORS], F32)
    nc.vector.tensor_add(out=tsum, in0=q2[:, 0, :], in1=q2[:, 1, :])
    t32 = const.tile([1, NPAD], F32)
    nc.vector.memset(t32, TBIG)
    nc.vector.scalar_tensor_tensor(
        out=t32[:, 0:NCOLORS], in0=tsum, scalar=EPS, in1=q2[:, 2, :],
        op0=ALU.add, op1=ALU.add,
    )
    thi16 = const.tile([1, NPAD], F16)
    nc.vector.tensor_scalar(out=thi16, in0=t32, scalar1=1.0, scalar2=None,
                            op0=ALU.mult)
    thi32 = const.tile([1, NPAD], F32)
    nc.vector.tensor_scalar(out=thi32, in0=thi16, scalar1=1.0, scalar2=None,
                            op0=ALU.mult)
    tlo32 = const.tile([1, NPAD], F32)
    nc.vector.tensor_tensor(out=tlo32, in0=t32, in1=thi32, op=ALU.subtract)

    # wtab [1, 6, 32]: b0 = 1.0 (x^2 rows), b1 = T_hi, b2 = T_lo,
    # b3..b5 = -2 q_c  (padded colors: T = TBIG, q = 0)
    wtab = const.tile([1, 6, NPAD], F32)
    nc.vector.memset(wtab, 0.0)
    nc.vector.memset(wtab[:, 0, :], 1.0)
    nc.vector.tensor_scalar(out=wtab[:, 1, :], in0=thi32, scalar1=1.0,
                            scalar2=None, op0=ALU.mult)
    nc.vector.tensor_scalar(out=wtab[:, 2, :], in0=tlo32, scalar1=1.0,
                            scalar2=None, op0=ALU.mult)
    nc.vector.tensor_scalar(out=wtab[:, 3:6, 0:NCOLORS], in0=qt, scalar1=-2.0,
                            scalar2=None, op0=ALU.mult)

    # broadcast to all partitions, then per-partition-block select
    wbig = const.tile([ROWS, 6 * NPAD], F32)
    nc.gpsimd.partition_broadcast(wbig, wtab.rearrange("p f k -> p (f k)"))
    wsel = const.tile([ROWS, NPAD], F32)
    nc.vector.memset(wsel, 0.0)
    for blk, (lo, hi) in enumerate(
        [(0, 48), (48, 49), (49, 50), (64, 80), (80, 96), (96, 112)]
    ):
        mge = const.tile([ROWS, 1], I32, tag=f"mge{blk}")
        nc.vector.tensor_scalar(
            out=mge, in0=iop112, scalar1=lo - 1, scalar2=None, op0=ALU.is_gt
        )
        mlt = const.tile([ROWS, 1], I32, tag=f"mlt{blk}")
        nc.vector.tensor_scalar(
            out=mlt, in0=iop112, scalar1=hi, scalar2=None, op0=ALU.is_lt
        )
        mm = const.tile([ROWS, 1], I32, tag=f"mm{blk}")
        nc.vector.tensor_tensor(out=mm, in0=mge, in1=mlt, op=ALU.mult)
        nc.vector.copy_predicated(
            out=wsel,
            mask=mm.to_broadcast([ROWS, NPAD]),
            data=wbig[:, blk * NPAD:(blk + 1) * NPAD],
        )

    # lhsT[p, 128j + g*8 + k] = sten[p, g] * wsel[p, 8j + k]   (k minor!)
    lhsT = const.tile([ROWS, NPASS * 128], F16)
    for j in range(NPASS):
        outv = lhsT[:, 128 * j:128 * (j + 1)].rearrange("p (g k) -> p g k", k=CPP)
        in0 = sten.unsqueeze(2).to_broadcast([ROWS, G, CPP])
        in1 = wsel[:, CPP * j:CPP * (j + 1)].unsqueeze(1).to_broadcast(
            [ROWS, G, CPP])
        nc.vector.tensor_tensor(out=outv, in0=in0, in1=in1, op=ALU.mult)
    emit_square(0)
    emit_square(1)
    hp.__exit__(None, None, None)




    z2 = z2pool.tile([128, 2, MMN], F32)       # pass 2, parity slots
    z3 = z3pool.tile([128, 2, MMN], F32)       # pass 3, parity slots
    pt = ptpool.tile([128, 2, 4, 128], F16)    # transposed survivors

    acc = const.tile([128, 2], F32)
    if probe is not None:
        nc.vector.memset(acc, 0.0)

    # PE p-state warm-up: harmless matmuls on the weight tile ramp the
    # tensor engine to full clock just before the first real matmuls
    for _ in range(12):
        zw = zpool.tile([128, MMN], F32, tag="z01")
        nc.tensor.matmul(out=zw, lhsT=lhsT[:, 0:128], rhs=lhsT[:, 0:MMN],
                         start=True, stop=True)

    collectors = []

    def emit_color_min(pair, stile_of):
        # transposes (PE) for `pair`, then fold the packed 8 colors (DVE)
        stile = stile_of[pair]
        for par in range(2):
            for chb in range(4):
                nc.tensor.transpose(
                    out=pt[:, par, chb, :],
                    in_=stile[:, par, 128 * chb:128 * (chb + 1)],
                    identity=id128,
                )
        if pair % 8 == 0:
            coll_new = collp.tile([128, 8, 128], F16, tag="coll")
            collectors.append(coll_new)
        coll = collectors[-1]
        ptv = pt.rearrange("p q c (g k) -> p q c g k", k=CPP)
        outv = coll[:, pair % 8, :].rearrange("p (q c g) -> p q c g", q=2, c=4)
        # packed fp16 min tree: one PSUM half is copied out first so every
        # TT sees at most one PSUM operand; packed fp16 runs at 2x
        u = t1pool.tile([128, 2, 4, G, 4], F16, tag="u")
        nc.vector.tensor_copy(out=u, in_=ptv[:, :, :, :, 4:8])
        t1 = t1pool.tile([128, 2, 4, G, 4], F16, tag="t1")
        nc.vector.tensor_tensor(out=t1, in0=ptv[:, :, :, :, 0:4], in1=u,
                                op=ALU.min)
        t2 = t1pool.tile([128, 2, 4, G, 2], F16, tag="t2")
        nc.vector.tensor_tensor(out=t2, in0=t1[:, :, :, :, 0:2],
                                in1=t1[:, :, :, :, 2:4], op=ALU.min)
        nc.vector.tensor_tensor(out=outv, in0=t2[:, :, :, :, 0],
                                in1=t2[:, :, :, :, 1], op=ALU.min)

    def emit_collector_finish(r):
        coll = collectors[r]
        nc.vector.tensor_scalar(
            out=coll, in0=coll, scalar1=0.0, scalar2=None, op0=ALU.max
        )
        scratch = sqp.tile([128, 8 * 128], F16, tag="sq")
        nc.scalar.activation(
            out=scratch, in_=coll.rearrange("p a b -> p (a b)"),
            func=ACTF.Sqrt, accum_out=acc[:, r:r + 1],
        )

    # ---------------- main loop -----------------------------------------
    stile_of = {}
    for pair in range(NPAIR):
        slab = pair // 4
        rhs = rhs_bufs[slab % 3]
        s16 = spool.tile([128, 2, 2, MMN], F16, tag="s16")
        for par in range(2):
            st = pair * 2 + par
            t = st % STS
            rsl = rhs[:, t * MMN:(t + 1) * MMN]
            for j in range(NPASS):
                if j < 2:
                    zt = zpool.tile([128, MMN], F32, tag="z01")
                elif j == 2:
                    zt = z2[:, par, :]
                else:
                    zt = z3[:, par, :]
                nc.tensor.matmul(
                    out=zt,
                    lhsT=lhsT[:, 128 * j:128 * (j + 1)],
                    rhs=rsl,
                    start=True,
                    stop=True,
                )
                if j < 2:
                    nc.scalar.activation(
                        out=s16[:, par, j, :], in_=zt, func=ACTF.Relu
                    )
        # transposes + color fold of an older pair ride here so the PE
        # never waits on the (deep) min pipeline
        if probe != "pe_only" and pair >= 3:
            emit_color_min(pair - 3, stile_of)
            stile_of.pop(pair - 3)

        if probe == "pe_only":
            continue

        # ScalarE also converts pass 2 (pair-merged); DVE folds the three
        # converted passes at fp16 2x and chains pass 3 from PSUM
        s2 = s2pool.tile([128, 2, MMN], F16, tag="s2")
        nc.scalar.activation(out=s2, in_=z2, func=ACTF.Relu)
        m1 = mpool.tile([128, 2, MMN], F16, tag="m1")
        nc.vector.tensor_tensor(
            out=m1, in0=s16[:, :, 0, :], in1=s16[:, :, 1, :], op=ALU.min
        )
        m2 = cpool.tile([128, 2, MMN], F16, tag="m2")
        nc.vector.tensor_tensor(out=m2, in0=m1, in1=s2, op=ALU.min)
        stile = stpool.tile([128, 2, MMN], F16, tag="stile")
        nc.vector.tensor_tensor(out=stile, in0=m2, in1=z3, op=ALU.min)
        stile_of[pair] = stile

        if 3 <= pair <= 6:
            emit_square(2, quarter=pair - 3)
        elif 7 <= pair <= 10:
            emit_square(3, quarter=pair - 7)


    if probe != "pe_only":
        for p in (NPAIR - 3, NPAIR - 2, NPAIR - 1):
            emit_color_min(p, stile_of)
        emit_collector_finish(0)
        emit_collector_finish(1)

    nc.sync.dma_start(out=out_d.ap(), in_=acc)
    ctx.close()


_CACHE = {}


def _get_program(probe=None):
    key = ("prog", probe)
    if key not in _CACHE:
        _CACHE[key] = _build_program(probe)
    return _CACHE[key]


def _prep_x(adv_patch):
    # device layout per slab: rows 0..1 ones, 2..15 zero, 16..63 x(c*16+g)
    x = (
        np.asarray(adv_patch, dtype=np.float32)
        .reshape(B, C, NSLAB, G, NFREE)
        .transpose(0, 2, 1, 3, 4)
        .reshape(B, NSLAB, 48, NFREE)
        .astype(np.float16)
    )
    xd = np.zeros((B, NSLAB, 64, NFREE), dtype=np.float16)
    xd[:, :, 0:2, :] = np.float16(1.0)
    xd[:, :, 16:64, :] = x
    return np.ascontiguousarray(xd)


def kernel(adv_patch: np.ndarray, printability: np.ndarray) -> np.ndarray:
    xd = _prep_x(adv_patch)
    p = np.ascontiguousarray(printability, dtype=np.float32)
    nc = _get_program()
    in_maps = [{"x": xd[b], "p": p} for b in range(B)]
    res = run_bass_kernel_spmd(nc, in_maps, core_ids=list(range(B)))
    total = np.float64(0.0)
    for r in res.results:
        total += r["out"].astype(np.float64).sum()
    return np.float32(total / (B * C * H * W))


def profile_once(inputs, trace_cores=None):
    xd = _prep_x(inputs["adv_patch"])
    p = np.ascontiguousarray(inputs["printability"], dtype=np.float32)
    nc = _get_program()
    in_maps = [{"x": xd[b], "p": p} for b in range(B)]
    try:
        res = run_bass_kernel_spmd(
            nc, in_maps, core_ids=list(range(B)), trace=True,
            trace_cores=trace_cores,
        )
        if res.instructions_and_trace is not None:
            print("trace:", res.instructions_and_trace[1])
        return res.exec_time_ns
    except Exception as e:
        print("profile_once failed:", e)
        return None


# revision 44
# speedup vs baseline: 1.0231x; 1.0231x over previous
"""NPS (non-printability score) kernel for Trainium2, 8-core data-parallel.

Math: for each pixel x (3 channels), distance to each of 30 printability
colors p_k is  d2_k = sum_c (x_c - p_c + 1e-6)^2 + 1e-6.  The score is
sum over pixels of sqrt(min_k d2_k), divided by adv_patch.size.

With q = p - 1e-6:  d2_k = S + (-2 x.q_k) + (T_k + 1e-6) where S = sum
x_c^2, T_k = |q_k|^2.  fp16 matmuls (1 PE cycle/column vs fp32's 4)
compute d2 for 8 colors x 16 pixel groups per 512-column pass; 4 passes
cover the 32 (padded) colors.  T rides in as two "ones"-row weights
(split hi/lo over two fp16 rows to kill weight-rounding error); the
ones rows and x rows arrive in a single HBM load per slab.

Post-matmul min funnel (z is fp32 in PSUM - TRN2 matmuls can't narrow -
and the funnel is boxed in by four hardware rules: tensor_tensor may
read at most one PSUM operand, GPSIMD has no PSUM access and no min op,
DMA cannot touch PSUM, and engine operands must start on 32-aligned
partitions.  Every z value therefore crosses PSUM->SBUF exactly once,
split across the two engines that can do it):
  - ScalarE: passes 0,1 (per-slot ops on a 3-bank rotation so PSUM
    frees early) and pass 2 (pair-merged) -> fp16 SBUF via Relu, which
    also clamps fp16-rounding negatives.
  - DVE: two packed-fp16 mins (2x rate) fold the converted passes, one
    mixed min chains pass 3 straight from PSUM.
  - PE transposes the survivor (fp16; colors k land packed innermost in
    each transposed block since lhsT columns are g*8+k); DVE folds the
    8 colors with a copy + packed-fp16 min tree (one PSUM half is
    copied out first so every TT sees at most one PSUM operand).
  - Per-pixel minima collect into a [128, 8*128] tile per 8 pairs; a
    tensor_scalar max-0 clamp and one ScalarE sqrt+accumulate per
    collector finish the job.  GPSIMD computes the x^2 rhs rows
    (Multiply is in its op set); a dozen warm-up matmuls ahead of the
    first real one hold the PE p-state at full clock.

PSUM (8 banks): z01 rotation 3 + z2 [128,2,512] 2 + z3 [128,2,512] 2
+ pt transpose buffer 1.

Sharding: batch dim (8 images) -> 8 NeuronCores, printability replicated.
"""

import numpy as np

import concourse.bass as bass
import concourse.bacc as bacc
import concourse.tile as tile
import concourse.mybir as mybir
from concourse.bass_utils import run_bass_kernel_spmd

F32 = mybir.dt.float32
F16 = mybir.dt.float16
I32 = mybir.dt.int32
ALU = mybir.AluOpType
ACTF = mybir.ActivationFunctionType

B, C, H, W = 8, 3, 512, 512
NCOLORS = 30
NPAD = 32            # colors padded to 32
NPASS = 4            # color passes, 8 colors each
CPP = 8              # colors per pass
G = 16               # pixel groups per matmul column block
MMN = 512            # matmul moving free dim (one fp32 PSUM bank)
NFREE = 4096         # per-partition free size of one slab
NSLAB = 4            # 4 slabs x 16 groups x 4096 = 262144 pixels/core
STS = NFREE // MMN   # supertiles per slab = 8
NPAIR = NSLAB * STS // 2   # 16 parity pairs
X0 = 64              # x rows base in rhs (rows 50..63 are zero pad)
ONES0 = 48           # two ones rows (T_hi / T_lo weights)
ROWS = 112
EPS = 1e-6
TBIG = 60000.0       # padded-color T: huge but finite in fp16
_WSEL_BLOCKS = [(0, 48), (48, 49), (49, 50), (64, 80), (80, 96), (96, 112)]


def _build_program(probe=None):
    nc = bacc.Bacc(
        "TRN2",
        target_bir_lowering=False,
        debug=False,
        enable_asserts=False,
        num_devices=B,
    )
    # x slab rows: 0..1 ones, 2..15 zero pad, 16..63 x (c*16+g)
    x_d = nc.dram_tensor("x", [NSLAB, 64, NFREE], F16, kind="ExternalInput")
    p_d = nc.dram_tensor("p", [NCOLORS, C], F32, kind="ExternalInput")
    out_d = nc.dram_tensor("out", [128, 2], F32, kind="ExternalOutput")

    with tile.TileContext(nc) as tc:
        _body(tc, nc, x_d, p_d, out_d, probe)
    nc.compile()
    return nc


def _body(tc, nc, x_d, p_d, out_d, probe=None):
    import contextlib

    ctx = contextlib.ExitStack()
    const = ctx.enter_context(tc.tile_pool(name="const", bufs=1))
    spool = ctx.enter_context(tc.tile_pool(name="spool", bufs=4))
    mpool = ctx.enter_context(tc.tile_pool(name="mpool", bufs=4))
    cpool = ctx.enter_context(tc.tile_pool(name="cpool", bufs=4))
    s2pool = ctx.enter_context(tc.tile_pool(name="s2pool", bufs=4))
    stpool = ctx.enter_context(tc.tile_pool(name="stpool", bufs=5))
    collp = ctx.enter_context(tc.tile_pool(name="collp", bufs=2))
    t1pool = ctx.enter_context(tc.tile_pool(name="t1pool", bufs=3))
    sqp = ctx.enter_context(tc.tile_pool(name="sqp", bufs=2))
    zpool = ctx.enter_context(tc.tile_pool(name="zpool", bufs=3, space="PSUM"))
    z2pool = ctx.enter_context(tc.tile_pool(name="z2pool", bufs=1, space="PSUM"))
    z3pool = ctx.enter_context(tc.tile_pool(name="z3pool", bufs=1, space="PSUM"))
    ptpool = ctx.enter_context(tc.tile_pool(name="ptpool", bufs=1, space="PSUM"))

    # ---------------- preamble: constants -------------------------------
    czero = const.tile([128, 1], F32)
    nc.vector.memset(czero, 0.0)
    nc.const_aps.aps[(F32, 0.0)] = czero[:]


    psbt = const.tile([1, C, NCOLORS], F32)
    hp = tc.high_priority()
    hp.__enter__()
    nc.sync.dma_start(out=psbt, in_=p_d.ap().transpose([1, 0]).unsqueeze(0))
    # rhs buffers: rows 0..47 squares, 48..49 ones + 50..63 pad (row 52
    # carries the fp16 printability table in slab 0) + 64..111 x
    rhs_bufs = []
    for i in range(3):
        rhs = const.tile([ROWS, NFREE], F16, tag=f"rhs{i}")
        rhs_bufs.append(rhs)
    for s in range(NSLAB):
        eng = nc.sync if s % 2 == 0 else nc.scalar
        eng.dma_start(out=rhs_bufs[s % 3][ONES0:ROWS, :], in_=x_d.ap()[s])

    # squares all on GPSIMD (Multiply is in its supported op set),
    # in quarters so nothing blocks the Pool stream for long
    def emit_square(s, quarter=None):
        # slab 0 runs on DVE: the Pool list-scheduler orders its stream by
        # its own heuristic and reliably starves the first slab otherwise
        eng = nc.vector if s == 0 else nc.gpsimd
        rhs = rhs_bufs[s % 3]
        qs = [quarter] if quarter is not None else range(4)
        for q in qs:
            q0 = q * (NFREE // 4)
            sl = slice(q0, q0 + NFREE // 4)
            eng.tensor_tensor(
                out=rhs[0:48, sl], in0=rhs[X0:ROWS, sl], in1=rhs[X0:ROWS, sl],
                op=ALU.mult,
            )


    # identity 128x128 fp16 for PE transpose
    iop128 = const.tile([128, 1], I32)
    nc.gpsimd.iota(iop128, pattern=[[0, 1]], base=0, channel_multiplier=1)
    iof128 = const.tile([128, 128], I32)
    nc.gpsimd.iota(iof128, pattern=[[1, 128]], base=0, channel_multiplier=0)
    id128 = const.tile([128, 128], F16)
    nc.vector.tensor_tensor(
        out=id128, in0=iof128, in1=iop128.to_broadcast([128, 128]), op=ALU.is_equal
    )

    # sten[p, g] = ((p & 15) == g); ones rows 48..49 forced to 1 for all g
    iop112 = const.tile([ROWS, 1], I32)
    nc.gpsimd.iota(iop112, pattern=[[0, 1]], base=0, channel_multiplier=1)
    pm112 = const.tile([ROWS, 1], I32)
    nc.vector.tensor_scalar(
        out=pm112, in0=iop112, scalar1=15, scalar2=None, op0=ALU.bitwise_and
    )
    iof16 = const.tile([ROWS, G], I32)
    nc.gpsimd.iota(iof16, pattern=[[1, G]], base=0, channel_multiplier=0)
    sten = const.tile([ROWS, G], F32)
    nc.vector.tensor_tensor(
        out=sten, in0=iof16, in1=pm112.to_broadcast([ROWS, G]), op=ALU.is_equal
    )
    mo_ge = const.tile([ROWS, 1], I32)
    nc.vector.tensor_scalar(out=mo_ge, in0=iop112, scalar1=ONES0 - 1,
                            scalar2=None, op0=ALU.is_gt)
    mo_lt = const.tile([ROWS, 1], I32)
    nc.vector.tensor_scalar(out=mo_lt, in0=iop112, scalar1=ONES0 + 2,
                            scalar2=None, op0=ALU.is_lt)
    mo = const.tile([ROWS, 1], F32)
    nc.vector.tensor_tensor(out=mo, in0=mo_ge, in1=mo_lt, op=ALU.mult)
    nc.vector.tensor_tensor(out=sten, in0=sten,
                            in1=mo.to_broadcast([ROWS, G]), op=ALU.max)

    # per-partition block-select masks for the weight table (input-
    # independent: computed while the printability DMA is in flight)
    blk_masks = []
    for blk, (lo, hi) in enumerate(_WSEL_BLOCKS):
        mge = const.tile([ROWS, 1], I32, tag=f"mge{blk}")
        nc.vector.tensor_scalar(
            out=mge, in0=iop112, scalar1=lo - 1, scalar2=None, op0=ALU.is_gt
        )
        mlt = const.tile([ROWS, 1], I32, tag=f"mlt{blk}")
        nc.vector.tensor_scalar(
            out=mlt, in0=iop112, scalar1=hi, scalar2=None, op0=ALU.is_lt
        )
        mm = const.tile([ROWS, 1], I32, tag=f"mm{blk}")
        nc.vector.tensor_tensor(out=mm, in0=mge, in1=mlt, op=ALU.mult)
        blk_masks.append(mm)

    # ---------------- preamble: weight table ----------------------------
    # q = p - eps;  T_k = |q_k|^2 + eps  (split hi/lo over two fp16 rows)
    qt = const.tile([1, C, NCOLORS], F32)
    nc.vector.tensor_scalar(out=qt, in0=psbt, scalar1=-EPS, scalar2=None,
                            op0=ALU.add)
    q2 = const.tile([1, C, NCOLORS], F32)
    nc.vector.tensor_tensor(out=q2, in0=qt, in1=qt, op=ALU.mult)
    tsum = const.tile([1, NCOLORS], F32)
    nc.vector.tensor_add(out=tsum, in0=q2[:, 0, :], in1=q2[:, 1, :])
    t32 = const.tile([1, NPAD], F32)
    nc.vector.memset(t32, TBIG)
    nc.vector.scalar_tensor_tensor(
        out=t32[:, 0:NCOLORS], in0=tsum, scalar=EPS, in1=q2[:, 2, :],
        op0=ALU.add, op1=ALU.add,
    )
    thi16 = const.tile([1, NPAD], F16)
    nc.vector.tensor_scalar(out=thi16, in0=t32, scalar1=1.0, scalar2=None,
                            op0=ALU.mult)
    thi32 = const.tile([1, NPAD], F32)
    nc.vector.tensor_scalar(out=thi32, in0=thi16, scalar1=1.0, scalar2=None,
                            op0=ALU.mult)
    tlo32 = const.tile([1, NPAD], F32)
    nc.vector.tensor_tensor(out=tlo32, in0=t32, in1=thi32, op=ALU.subtract)

    # wtab [1, 6, 32]: b0 = 1.0 (x^2 rows), b1 = T_hi, b2 = T_lo,
    # b3..b5 = -2 q_c  (padded colors: T = TBIG, q = 0)
    wtab = const.tile([1, 6, NPAD], F32)
    nc.vector.memset(wtab, 0.0)
    nc.vector.memset(wtab[:, 0, :], 1.0)
    nc.vector.tensor_scalar(out=wtab[:, 1, :], in0=thi32, scalar1=1.0,
                            scalar2=None, op0=ALU.mult)
    nc.vector.tensor_scalar(out=wtab[:, 2, :], in0=tlo32, scalar1=1.0,
                            scalar2=None, op0=ALU.mult)
    nc.vector.tensor_scalar(out=wtab[:, 3:6, 0:NCOLORS], in0=qt, scalar1=-2.0,
                            scalar2=None, op0=ALU.mult)

    # broadcast to all partitions, then per-partition-block select
    wbig = const.tile([ROWS, 6 * NPAD], F32)
    nc.gpsimd.partition_broadcast(wbig, wtab.rearrange("p f k -> p (f k)"))
    wsel = const.tile([ROWS, NPAD], F32)
    nc.vector.memset(wsel, 0.0)
    for blk, (lo, hi) in enumerate(_WSEL_BLOCKS):
        nc.vector.copy_predicated(
            out=wsel,
            mask=blk_masks[blk].to_broadcast([ROWS, NPAD]),
            data=wbig[:, blk * NPAD:(blk + 1) * NPAD],
        )

    # lhsT[p, 128j + g*8 + k] = sten[p, g] * wsel[p, 8j + k]   (k minor!)
    lhsT = const.tile([ROWS, NPASS * 128], F16)
    for j in range(NPASS):
        outv = lhsT[:, 128 * j:128 * (j + 1)].rearrange("p (g k) -> p g k", k=CPP)
        in0 = sten.unsqueeze(2).to_broadcast([ROWS, G, CPP])
        in1 = wsel[:, CPP * j:CPP * (j + 1)].unsqueeze(1).to_broadcast(
            [ROWS, G, CPP])
        nc.vector.tensor_tensor(out=outv, in0=in0, in1=in1, op=ALU.mult)
    emit_square(0)
    emit_square(1)
    hp.__exit__(None, None, None)




    z2 = z2pool.tile([128, 2, MMN], F32)       # pass 2, parity slots
    z3 = z3pool.tile([128, 2, MMN], F32)       # pass 3, parity slots
    pt = ptpool.tile([128, 2, 4, 128], F16)    # transposed survivors

    acc = const.tile([128, 2], F32)
    if probe is not None:
        nc.vector.memset(acc, 0.0)

    # PE p-state warm-up: harmless matmuls on the weight tile ramp the
    # tensor engine to full clock just before the first real matmuls
    for _ in range(8):
        zw = zpool.tile([128, MMN], F32, tag="z01")
        nc.tensor.matmul(out=zw, lhsT=lhsT[:, 0:128], rhs=lhsT[:, 0:MMN],
                         start=True, stop=True)

    collectors = []

    def emit_color_min(pair, stile_of):
        # transposes (PE) for `pair`, then fold the packed 8 colors (DVE)
        stile = stile_of[pair]
        for par in range(2):
            for chb in range(4):
                nc.tensor.transpose(
                    out=pt[:, par, chb, :],
                    in_=stile[:, par, 128 * chb:128 * (chb + 1)],
                    identity=id128,
                )
        if pair % 8 == 0:
            coll_new = collp.tile([128, 8, 128], F16, tag="coll")
            collectors.append(coll_new)
        coll = collectors[-1]
        ptv = pt.rearrange("p q c (g k) -> p q c g k", k=CPP)
        outv = coll[:, pair % 8, :].rearrange("p (q c g) -> p q c g", q=2, c=4)
        # packed fp16 min tree: one PSUM half is copied out first so every
        # TT sees at most one PSUM operand; packed fp16 runs at 2x
        u = t1pool.tile([128, 2, 4, G, 4], F16, tag="u")
        nc.vector.tensor_copy(out=u, in_=ptv[:, :, :, :, 4:8])
        t1 = t1pool.tile([128, 2, 4, G, 4], F16, tag="t1")
        nc.vector.tensor_tensor(out=t1, in0=ptv[:, :, :, :, 0:4], in1=u,
                                op=ALU.min)
        t2 = t1pool.tile([128, 2, 4, G, 2], F16, tag="t2")
        nc.vector.tensor_tensor(out=t2, in0=t1[:, :, :, :, 0:2],
                                in1=t1[:, :, :, :, 2:4], op=ALU.min)
        nc.vector.tensor_tensor(out=outv, in0=t2[:, :, :, :, 0],
                                in1=t2[:, :, :, :, 1], op=ALU.min)

    def emit_collector_finish(r):
        coll = collectors[r]
        nc.vector.tensor_scalar(
            out=coll, in0=coll, scalar1=0.0, scalar2=None, op0=ALU.max
        )
        scratch = sqp.tile([128, 8 * 128], F16, tag="sq")
        nc.scalar.activation(
            out=scratch, in_=coll.rearrange("p a b -> p (a b)"),
            func=ACTF.Sqrt, accum_out=acc[:, r:r + 1],
        )

    # ---------------- main loop -----------------------------------------
    stile_of = {}
    for pair in range(NPAIR):
        slab = pair // 4
        rhs = rhs_bufs[slab % 3]
        s16 = spool.tile([128, 2, 2, MMN], F16, tag="s16")
        for par in range(2):
            st = pair * 2 + par
            t = st % STS
            rsl = rhs[:, t * MMN:(t + 1) * MMN]
            for j in range(NPASS):
                if j < 2:
                    zt = zpool.tile([128, MMN], F32, tag="z01")
                elif j == 2:
                    zt = z2[:, par, :]
                else:
                    zt = z3[:, par, :]
                nc.tensor.matmul(
                    out=zt,
                    lhsT=lhsT[:, 128 * j:128 * (j + 1)],
                    rhs=rsl,
                    start=True,
                    stop=True,
                )
                if j < 2:
                    nc.scalar.activation(
                        out=s16[:, par, j, :], in_=zt, func=ACTF.Relu
                    )
        # transposes + color fold of an older pair ride here so the PE
        # never waits on the (deep) min pipeline
        if probe != "pe_only" and pair >= 2:
            emit_color_min(pair - 2, stile_of)
            stile_of.pop(pair - 2)

        if probe == "pe_only":
            continue

        # ScalarE also converts pass 2 (pair-merged); DVE folds the three
        # converted passes at fp16 2x and chains pass 3 from PSUM.  On the
        # last pairs ScalarE converts pass 3 too - it idles there waiting
        # for the tail - turning the 1x chain into a 2x fp16 fold.
        s2 = s2pool.tile([128, 2, MMN], F16, tag="s2")
        nc.scalar.activation(out=s2, in_=z2, func=ACTF.Relu)
        m1 = mpool.tile([128, 2, MMN], F16, tag="m1")
        nc.vector.tensor_tensor(
            out=m1, in0=s16[:, :, 0, :], in1=s16[:, :, 1, :], op=ALU.min
        )
        m2 = cpool.tile([128, 2, MMN], F16, tag="m2")
        nc.vector.tensor_tensor(out=m2, in0=m1, in1=s2, op=ALU.min)
        stile = stpool.tile([128, 2, MMN], F16, tag="stile")
        if pair >= NPAIR - 2:
            s3 = s2pool.tile([128, 2, MMN], F16, tag="s3")
            nc.scalar.activation(out=s3, in_=z3, func=ACTF.Relu)
            nc.vector.tensor_tensor(out=stile, in0=m2, in1=s3, op=ALU.min)
        else:
            nc.vector.tensor_tensor(out=stile, in0=m2, in1=z3, op=ALU.min)
        stile_of[pair] = stile

        if 3 <= pair <= 6:
            emit_square(2, quarter=pair - 3)
        elif 7 <= pair <= 10:
            emit_square(3, quarter=pair - 7)


    if probe != "pe_only":
        for p in (NPAIR - 2, NPAIR - 1):
            emit_color_min(p, stile_of)
        emit_collector_finish(0)
        emit_collector_finish(1)

    nc.sync.dma_start(out=out_d.ap(), in_=acc)
    ctx.close()


_CACHE = {}


def _get_program(probe=None):
    key = ("prog", probe)
    if key not in _CACHE:
        _CACHE[key] = _build_program(probe)
    return _CACHE[key]


def _prep_x(adv_patch):
    # device layout per slab: rows 0..1 ones, 2..15 zero, 16..63 x(c*16+g)
    x = (
        np.asarray(adv_patch, dtype=np.float32)
        .reshape(B, C, NSLAB, G, NFREE)
        .transpose(0, 2, 1, 3, 4)
        .reshape(B, NSLAB, 48, NFREE)
        .astype(np.float16)
    )
    xd = np.zeros((B, NSLAB, 64, NFREE), dtype=np.float16)
    xd[:, :, 0:2, :] = np.float16(1.0)
    xd[:, :, 16:64, :] = x
    return np.ascontiguousarray(xd)


def kernel(adv_patch: np.ndarray, printability: np.ndarray) -> np.ndarray:
    xd = _prep_x(adv_patch)
    p = np.ascontiguousarray(printability, dtype=np.float32)
    nc = _get_program()
    in_maps = [{"x": xd[b], "p": p} for b in range(B)]
    res = run_bass_kernel_spmd(nc, in_maps, core_ids=list(range(B)))
    total = np.float64(0.0)
    for r in res.results:
        total += r["out"].astype(np.float64).sum()
    return np.float32(total / (B * C * H * W))


def profile_once(inputs, trace_cores=None):
    xd = _prep_x(inputs["adv_patch"])
    p = np.ascontiguousarray(inputs["printability"], dtype=np.float32)
    nc = _get_program()
    in_maps = [{"x": xd[b], "p": p} for b in range(B)]
    try:
        res = run_bass_kernel_spmd(
            nc, in_maps, core_ids=list(range(B)), trace=True,
            trace_cores=trace_cores,
        )
        if res.instructions_and_trace is not None:
            print("trace:", res.instructions_and_trace[1])
        return res.exec_time_ns
    except Exception as e:
        print("profile_once failed:", e)
        return None
